# revision 17
# baseline (speedup 1.0000x reference)
"""DeepSeek-V2-style MoE kernel for 8 Trainium2 NeuronCores — sparse dispatch.

Sharding strategy:
  - Routing (gate matmul + softmax + group-limited top-2) runs on the host in
    fp32 numpy, mirroring the reference exactly. Only the top-2 experts per
    token contribute (combine weights of the rest are exactly zero), so each
    core computes its expert on just the tokens routed to it (gathered and
    zero-padded to a fixed capacity C=640; ~554 max at T=2048, K=2).
  - Core n runs routed expert n on its gathered tokens, scaled by the
    per-token combine weight, producing a [C, H] output the host scatters.
  - The always-on shared expert is 2D-sharded: 4-way over tokens x 2-way over
    the FS=2816 intermediate dim, so each core handles 512 tokens x 1408
    channels — the same shapes as the routed phase. Host adds core pairs.
  - If an expert ever exceeds capacity (not the case for the fixed harness
    input), the overflow tokens are computed exactly on the host in fp32.

All matmuls run in bf16 with fp32 PSUM accumulation, weight-stationary
(the PE pulls LDWEIGHTS ahead of in-flight matmuls, so per-(fb,ko) weight
loads hide under the 512-cycle token streams).
"""

from contextlib import ExitStack

import numpy as np
import ml_dtypes

import concourse.bass as bass
import concourse.tile as tile
from concourse import bacc, mybir
from concourse.bass_utils import run_bass_kernel_spmd

BF16 = ml_dtypes.bfloat16
F32 = np.float32

P = 128
B, S, H, F, FS, NEXP = 2, 1024, 2048, 1408, 2816, 8
T = B * S                      # 2048 tokens
TOP_K = 2
N_GROUP = 4
TOPK_GROUP = 2
KO = H // P                    # 16 contraction chunks over H
FBN = F // P                   # 11 intermediate-dim blocks of 128
C = 640                        # routed token capacity per expert
CB = C // P                    # 5 routed token blocks
TS = T // 4                    # 512 shared-expert tokens per core
SB = TS // P                   # 4 shared token blocks
HC = H // 512                  # 4 output chunks of 512

_ALU = mybir.AluOpType
_ACTF = mybir.ActivationFunctionType
_F32 = mybir.dt.float32
_BF16 = mybir.dt.bfloat16

_CACHED_NC = None


def _build_body(ctx, tc):
    nc = tc.nc
    xg_d = nc.dram_tensor("xg", [P, KO, C], _BF16, kind="ExternalInput").ap()
    xs_d = nc.dram_tensor("xs", [P, KO, TS], _BF16, kind="ExternalInput").ap()
    wg_d = nc.dram_tensor("wg", [P, FBN, KO, P], _BF16, kind="ExternalInput").ap()
    wu_d = nc.dram_tensor("wu", [P, FBN, KO, P], _BF16, kind="ExternalInput").ap()
    sg_d = nc.dram_tensor("sg", [P, FBN, KO, P], _BF16, kind="ExternalInput").ap()
    su_d = nc.dram_tensor("su", [P, FBN, KO, P], _BF16, kind="ExternalInput").ap()
    dw_d = nc.dram_tensor("dw", [P, FBN, H], _BF16, kind="ExternalInput").ap()
    sd_d = nc.dram_tensor("sd", [P, FBN, H], _BF16, kind="ExternalInput").ap()
    wr_d = nc.dram_tensor("wr", [P, CB], _F32, kind="ExternalInput").ap()
    outr_d = nc.dram_tensor("outr", [C, H], _F32, kind="ExternalOutput").ap()
    outs_d = nc.dram_tensor("outs", [TS, H], _F32, kind="ExternalOutput").ap()

    consts = ctx.enter_context(tc.tile_pool(name="consts", bufs=1))
    xpool = ctx.enter_context(tc.tile_pool(name="xpool", bufs=1))
    wpool = ctx.enter_context(tc.tile_pool(name="wpool", bufs=2))
    dpool = ctx.enter_context(tc.tile_pool(name="dpool", bufs=1))
    apool = ctx.enter_context(tc.tile_pool(name="apool", bufs=1))
    spool = ctx.enter_context(tc.tile_pool(name="spool", bufs=2))
    opool = ctx.enter_context(tc.tile_pool(name="opool", bufs=2))
    mmp = ctx.enter_context(tc.tile_pool(name="mmp", bufs=1, space="PSUM"))

    # Bulk loads go on the scalar (Activation) HWDGE queue so the sync
    # queue's first transfer is the fb0 gate-weight tile the PE waits on.
    wr_sb = consts.tile([P, CB], _F32)
    nc.scalar.dma_start(wr_sb[:], wr_d[:])

    xg_sb = xpool.tile([P, KO, C], _BF16)
    nc.scalar.dma_start(xg_sb[:, 0:4, :], xg_d[:, 0:4, :])
    nc.scalar.dma_start(xg_sb[:, 4:KO, :], xg_d[:, 4:KO, :])
    # down-proj weights: dw and sd share one slot (sd's DMA is deferred and
    # waits for the routed down phase to finish reading dw)
    dw_sb = dpool.tile([P, FBN, H], _BF16, tag="dwn", name="dw_sb")
    xs_sb = xpool.tile([P, KO, TS], _BF16)

    aT = apool.tile([P, FBN, C], _BF16)    # routed silu(g)*u, [f, tok]
    asT = apool.tile([P, FBN, TS], _BF16)  # shared silu(g)*u, [f, tok]

    FBG = 2                       # weight-stream DMA group: 2 fb per op
    GROUPS = [(0, 1)] + [(g, min(FBG, FBN - g)) for g in range(1, FBN, FBG)]

    def gu_phase(gsrc, usrc, x_sb, chunks, dst, after_group=None):
        """Gate/up projections + silu(g)*u for one FFN, weight-stationary.
        Weight tiles stream in 2-fb groups (1 MB DMA ops amortize the ~2us
        per-op completion latency); the first group is a single fb so the
        PE can start sooner. after_group: {group_idx: fn} hooks, used to
        slot bulk DMA triggers into the scalar queue behind early sigmoids
        (so their transfers stay clear of the startup weight stream)."""
        for gi, (g0, gsz) in enumerate(GROUPS):
            if after_group and gi in after_group:
                after_group[gi]()
            wg_t = wpool.tile([P, FBG, KO, P], _BF16, tag="wg", bufs=3,
                              name="wg_t")
            nc.sync.dma_start(wg_t[:, :gsz], gsrc[:, g0:g0 + gsz])
            wu_t = wpool.tile([P, FBG, KO, P], _BF16, tag="wu", bufs=3,
                              name="wu_t")
            nc.sync.dma_start(wu_t[:, :gsz], usrc[:, g0:g0 + gsz])
            for j in range(gsz):
                fb = g0 + j
                pgs = [mmp.tile([P, sz], _F32, tag=f"p{2*i}", bufs=2,
                                name=f"pg{i}")
                       for i, (_, sz) in enumerate(chunks)]
                pus = [mmp.tile([P, sz], _F32, tag=f"p{2*i+1}", bufs=2,
                                name=f"pu{i}")
                       for i, (_, sz) in enumerate(chunks)]
                for ko in range(KO):
                    for i, (o, sz) in enumerate(chunks):
                        nc.tensor.matmul(
                            pgs[i][:], wg_t[:, j, ko, :],
                            x_sb[:, ko, o:o + sz],
                            start=(ko == 0), stop=(ko == KO - 1),
                        )
                for ko in range(KO):
                    for i, (o, sz) in enumerate(chunks):
                        nc.tensor.matmul(
                            pus[i][:], wu_t[:, j, ko, :],
                            x_sb[:, ko, o:o + sz],
                            start=(ko == 0), stop=(ko == KO - 1),
                        )
                for i, (o, sz) in enumerate(chunks):
                    sg = spool.tile([P, sz], _F32, tag="sg", name="sg")
                    nc.scalar.activation(sg[:], pgs[i][:], _ACTF.Sigmoid)
                    nc.vector.tensor_tensor(sg[:], sg[:], pgs[i][:],
                                            _ALU.mult)
                    nc.vector.tensor_tensor(dst[:, fb, o:o + sz], sg[:],
                                            pus[i][:], _ALU.mult)

    def down_phase(a_sb, d_sb, ntb, out_d, scale, split_out=False):
        """Down-projection, activation-stationary (lhs = a[f, tok-block]),
        streaming the [f, H] weights as the moving operand."""
        for tb in range(ntb):
            tbs = slice(tb * P, (tb + 1) * P)
            pds = [mmp.tile([P, 512], _F32, tag=f"p{hc}", bufs=2,
                            name=f"pd{hc}") for hc in range(HC)]
            for fb in range(FBN):
                for hc in range(HC):
                    nc.tensor.matmul(
                        pds[hc][:], a_sb[:, fb, tbs],
                        d_sb[:, fb, hc * 512:(hc + 1) * 512],
                        start=(fb == 0), stop=(fb == FBN - 1),
                    )
            o = opool.tile([P, H], _F32, tag="o", bufs=3, name="o")
            for hc in range(HC):
                hs = slice(hc * 512, (hc + 1) * 512)
                if scale:
                    nc.vector.tensor_scalar_mul(o[:, hs], pds[hc][:],
                                                wr_sb[:, tb:tb + 1])
                else:
                    nc.vector.tensor_copy(o[:, hs], pds[hc][:])
                if split_out:
                    eng = nc.scalar if hc % 2 == 0 else nc.sync
                    eng.dma_start(out_d[tbs, hs], o[:, hs])
            if not split_out:
                eng = nc.scalar if tb % 2 == 0 else nc.sync
                eng.dma_start(out_d[tbs, :], o[:])

    r_chunks = [(0, 512), (512, C - 512)]
    s_chunks = [(0, 512)]
    gu_phase(wg_d, wu_d, xg_sb, r_chunks, aT, after_group={
        1: lambda: nc.scalar.dma_start(dw_sb[:], dw_d[:]),
        2: lambda: nc.scalar.dma_start(xs_sb[:], xs_d[:]),
    })
    down_phase(aT, dw_sb, CB, outr_d, scale=True)
    # sd reuses dw's SBUF slot: its DMA waits for the routed down phase's
    # last dw read, landing well before the shared down phase needs it
    sd_sb = dpool.tile([P, FBN, H], _BF16, tag="dwn", name="sd_sb")
    nc.scalar.dma_start(sd_sb[:], sd_d[:])
    gu_phase(sg_d, su_d, xs_sb, s_chunks, asT)
    down_phase(asT, sd_sb, SB, outs_d, scale=False, split_out=True)


def build_program():
    nc = bacc.Bacc("TRN2", target_bir_lowering=False, debug=False)
    with tile.TileContext(nc) as tc:
        with ExitStack() as ctx:
            _build_body(ctx, tc)
    nc.compile()
    return nc


def _get_nc():
    global _CACHED_NC
    if _CACHED_NC is None:
        _CACHED_NC = build_program()
    return _CACHED_NC


def _route(h, gate_weight):
    """Mirror of the reference's softmax + group-limited top-2, numpy fp32."""
    logits = (h @ gate_weight.T).astype(F32)
    m = logits.max(-1, keepdims=True)
    e = np.exp(logits - m)
    scores = e / e.sum(-1, keepdims=True)                     # [T, N]
    E = NEXP // N_GROUP
    gs = scores.reshape(T, N_GROUP, E).max(-1)                # [T, G]
    gidx = np.argsort(-gs, axis=1, kind="stable")[:, :TOPK_GROUP]
    gmask = np.zeros((T, N_GROUP), F32)
    np.put_along_axis(gmask, gidx, 1.0, axis=1)
    masked = np.where(np.repeat(gmask, E, axis=1) > 0, scores, 0.0)
    ti = np.argsort(-masked, axis=1, kind="stable")[:, :TOP_K]  # [T, K]
    tw = np.take_along_axis(masked, ti, axis=1)               # [T, K]
    return tw, ti


def _wslab(w, half=None):
    """[F', H] row-major weight -> [P, FBN, KO, P] bf16 lhsT layout."""
    m = w if half is None else w[half * F:(half + 1) * F]
    return np.ascontiguousarray(
        m.reshape(FBN, P, KO, P).transpose(3, 0, 2, 1).astype(BF16))


def _dslab(w, half=None):
    """[H, F'] down weight -> [P, FBN, H] bf16 (f-inner, fb, h') layout."""
    m = w if half is None else w[:, half * F:(half + 1) * F]
    return np.ascontiguousarray(
        m.T.reshape(FBN, P, H).transpose(1, 0, 2).astype(BF16))


def _xslab(hT, pad_to):
    """[H, t] f32 column-slice of tokens -> [P, KO, pad_to] bf16."""
    t = hT.shape[1]
    out = np.zeros((P, KO, pad_to), BF16)
    out[:, :, :t] = hT.reshape(KO, P, t).transpose(1, 0, 2).astype(BF16)
    return out


def prepare(inputs):
    h = np.asarray(inputs["hidden_states"], F32).reshape(T, H)
    hT = np.ascontiguousarray(h.T)                            # [H, T]
    tw, ti = _route(h, np.asarray(inputs["gate_weight"], F32))

    gate_w = np.asarray(inputs["gate_w"], F32)
    up_w = np.asarray(inputs["up_w"], F32)
    down_w = np.asarray(inputs["down_w"], F32)

    # shared-expert shards, built once and referenced by multiple cores
    sgh = [_wslab(np.asarray(inputs["sh_gate_w"], F32), hn) for hn in (0, 1)]
    suh = [_wslab(np.asarray(inputs["sh_up_w"], F32), hn) for hn in (0, 1)]
    sdh = [_dslab(np.asarray(inputs["sh_down_w"], F32), hn) for hn in (0, 1)]
    xsq = [_xslab(hT[:, q * TS:(q + 1) * TS], TS) for q in range(4)]

    in_maps, gathers, overflows = [], [], []
    for n in range(NEXP):
        sel = np.nonzero(ti == n)
        idx = sel[0]                                          # token ids
        w = tw[sel[0], sel[1]]                                # combine weights
        if len(idx) > C:
            overflows.append((n, idx[C:], w[C:]))
            idx, w = idx[:C], w[:C]
        gathers.append((idx, len(idx)))
        wr = np.zeros(C, F32)
        wr[:len(idx)] = w
        in_maps.append({
            "xg": _xslab(hT[:, idx], C),
            "xs": xsq[n // 2],
            "wg": _wslab(gate_w[n]),
            "wu": _wslab(up_w[n]),
            "sg": sgh[n % 2],
            "su": suh[n % 2],
            "dw": _dslab(down_w[n]),
            "sd": sdh[n % 2],
            "wr": np.ascontiguousarray(wr.reshape(CB, P).T),
        })
    return in_maps, gathers, overflows


def _silu(x):
    return x / (1.0 + np.exp(-x))


def run(inputs, trace=False, **kwargs):
    nc = _get_nc()
    in_maps, gathers, overflows = prepare(inputs)
    res = run_bass_kernel_spmd(
        nc, in_maps, core_ids=list(range(NEXP)), trace=trace, **kwargs
    )
    out = np.empty((T, H), F32)
    for q in range(4):
        out[q * TS:(q + 1) * TS] = (res.results[2 * q]["outs"]
                                    + res.results[2 * q + 1]["outs"])
    for n in range(NEXP):
        idx, cnt = gathers[n]
        out[idx] += res.results[n]["outr"][:cnt]
    for n, idx, w in overflows:   # exact host fallback, normally empty
        x = np.asarray(inputs["hidden_states"], F32).reshape(T, H)[idx]
        g = x @ np.asarray(inputs["gate_w"][n], F32).T
        u = x @ np.asarray(inputs["up_w"][n], F32).T
        out[idx] += ((_silu(g) * u) @ np.asarray(inputs["down_w"][n], F32).T
                     * w[:, None])
    return out.reshape(B, S, H), res


def kernel(**inputs):
    out, _ = run(inputs)
    return out


# revision 23
# speedup vs baseline: 1.0348x; 1.0348x over previous
"""DeepSeek-V2-style MoE kernel for 8 Trainium2 NeuronCores — sparse dispatch.

Sharding strategy:
  - Routing (gate matmul + softmax + group-limited top-2) runs on the host in
    fp32 numpy, mirroring the reference exactly. Only the top-2 experts per
    token contribute (combine weights of the rest are exactly zero), so each
    core computes its expert on just the tokens routed to it (gathered and
    zero-padded to a fixed capacity C=640; ~554 max at T=2048, K=2).
  - Core n runs routed expert n on its gathered tokens, scaled by the
    per-token combine weight, producing a [C, H] output the host scatters.
  - The always-on shared expert is 2D-sharded: 4-way over tokens x 2-way over
    the FS=2816 intermediate dim, so each core handles 512 tokens x 1408
    channels — the same shapes as the routed phase. Host adds core pairs.
  - If an expert ever exceeds capacity (not the case for the fixed harness
    input), the overflow tokens are computed exactly on the host in fp32.

All matmuls run in bf16 with fp32 PSUM accumulation, weight-stationary
(the PE pulls LDWEIGHTS ahead of in-flight matmuls, so per-(fb,ko) weight
loads hide under the 512-cycle token streams).
"""

from contextlib import ExitStack

import numpy as np
import ml_dtypes

import concourse.bass as bass
import concourse.tile as tile
from concourse import bacc, mybir
from concourse.bass_utils import run_bass_kernel_spmd

BF16 = ml_dtypes.bfloat16
F32 = np.float32

P = 128
B, S, H, F, FS, NEXP = 2, 1024, 2048, 1408, 2816, 8
T = B * S                      # 2048 tokens
TOP_K = 2
N_GROUP = 4
TOPK_GROUP = 2
KO = H // P                    # 16 contraction chunks over H
FBN = F // P                   # 11 intermediate-dim blocks of 128
C = 576                        # routed token capacity per expert (max count
                               # at T=2048,K=2 is ~554; host fallback beyond)
CB = 5                         # routed combine-weight blocks (640-padded)
TS = T // 4                    # 512 shared-expert tokens per core
SB = TS // P                   # 4 shared token blocks
HC = H // 512                  # 4 output chunks of 512

_ALU = mybir.AluOpType
_ACTF = mybir.ActivationFunctionType
_F32 = mybir.dt.float32
_BF16 = mybir.dt.bfloat16

_CACHED_NC = None


def _build_body(ctx, tc):
    nc = tc.nc
    xg_d = nc.dram_tensor("xg", [P, KO, C], _BF16, kind="ExternalInput").ap()
    xs_d = nc.dram_tensor("xs", [P, KO, TS], _BF16, kind="ExternalInput").ap()
    wg_d = nc.dram_tensor("wg", [P, FBN, KO, P], _BF16, kind="ExternalInput").ap()
    wu_d = nc.dram_tensor("wu", [P, FBN, KO, P], _BF16, kind="ExternalInput").ap()
    sg_d = nc.dram_tensor("sg", [P, FBN, KO, P], _BF16, kind="ExternalInput").ap()
    su_d = nc.dram_tensor("su", [P, FBN, KO, P], _BF16, kind="ExternalInput").ap()
    dw_d = nc.dram_tensor("dw", [P, FBN, H], _BF16, kind="ExternalInput").ap()
    sd_d = nc.dram_tensor("sd", [P, FBN, H], _BF16, kind="ExternalInput").ap()
    wr_d = nc.dram_tensor("wr", [P, CB], _F32, kind="ExternalInput").ap()
    outr_d = nc.dram_tensor("outr", [C, H], _F32, kind="ExternalOutput").ap()
    outs_d = nc.dram_tensor("outs", [TS, H], _F32, kind="ExternalOutput").ap()

    consts = ctx.enter_context(tc.tile_pool(name="consts", bufs=1))
    xpool = ctx.enter_context(tc.tile_pool(name="xpool", bufs=1))
    wpool = ctx.enter_context(tc.tile_pool(name="wpool", bufs=2))
    dpool = ctx.enter_context(tc.tile_pool(name="dpool", bufs=1))
    apool = ctx.enter_context(tc.tile_pool(name="apool", bufs=1))
    spool = ctx.enter_context(tc.tile_pool(name="spool", bufs=2))
    opool = ctx.enter_context(tc.tile_pool(name="opool", bufs=2))
    mmp = ctx.enter_context(tc.tile_pool(name="mmp", bufs=1, space="PSUM"))

    # Bulk loads go on the scalar (Activation) HWDGE queue so the sync
    # queue's first transfer is the fb0 gate-weight tile the PE waits on.
    wr_sb = consts.tile([P, CB], _F32)
    nc.scalar.dma_start(wr_sb[:], wr_d[:])

    xg_sb = xpool.tile([P, KO, C], _BF16)
    for i in range(4):
        ks = slice(i * 4, (i + 1) * 4)
        nc.scalar.dma_start(xg_sb[:, ks, :], xg_d[:, ks, :])
    # down-proj weights: dw and sd share one slot (sd's DMA is deferred and
    # waits for the routed down phase to finish reading dw)
    dw_sb = dpool.tile([P, FBN, H], _BF16, tag="dwn", name="dw_sb")
    xs_sb = xpool.tile([P, KO, TS], _BF16)

    aT = apool.tile([P, FBN, C], _BF16)    # routed silu(g)*u, [f, tok]
    asT = apool.tile([P, FBN, TS], _BF16)  # shared silu(g)*u, [f, tok]

    FBG = 2                       # weight-stream DMA group: 2 fb per op
    GROUPS = [(0, 1)] + [(g, min(FBG, FBN - g)) for g in range(1, FBN, FBG)]

    def gu_phase(gsrc, usrc, x_sb, chunks, dst, after_group=None):
        """Gate/up projections + silu(g)*u for one FFN, weight-stationary.
        Weight tiles stream in 2-fb groups (1 MB DMA ops amortize the ~2us
        per-op completion latency); the first group is a single fb so the
        PE can start sooner. after_group: {group_idx: fn} hooks, used to
        slot bulk DMA triggers into the scalar queue behind early sigmoids
        (so their transfers stay clear of the startup weight stream)."""
        for gi, (g0, gsz) in enumerate(GROUPS):
            if after_group and gi in after_group:
                after_group[gi]()
            wg_t = wpool.tile([P, FBG, KO, P], _BF16, tag="wg", bufs=3,
                              name="wg_t")
            wu_t = wpool.tile([P, FBG, KO, P], _BF16, tag="wu", bufs=3,
                              name="wu_t")
            if gi == 0:
                # halve the first ops so the PE's ko-loop starts sooner
                nc.sync.dma_start(wg_t[:, :gsz, 0:8], gsrc[:, g0:g0 + gsz, 0:8])
                nc.sync.dma_start(wu_t[:, :gsz, 0:8], usrc[:, g0:g0 + gsz, 0:8])
                nc.sync.dma_start(wg_t[:, :gsz, 8:KO], gsrc[:, g0:g0 + gsz, 8:KO])
                nc.sync.dma_start(wu_t[:, :gsz, 8:KO], usrc[:, g0:g0 + gsz, 8:KO])
            else:
                nc.sync.dma_start(wg_t[:, :gsz], gsrc[:, g0:g0 + gsz])
                nc.sync.dma_start(wu_t[:, :gsz], usrc[:, g0:g0 + gsz])
            for j in range(gsz):
                fb = g0 + j
                pgs = [mmp.tile([P, sz], _F32, tag=f"p{2*i}", bufs=2,
                                name=f"pg{i}")
                       for i, (_, sz) in enumerate(chunks)]
                pus = [mmp.tile([P, sz], _F32, tag=f"p{2*i+1}", bufs=2,
                                name=f"pu{i}")
                       for i, (_, sz) in enumerate(chunks)]
                for ko in range(KO):
                    for i, (o, sz) in enumerate(chunks):
                        nc.tensor.matmul(
                            pgs[i][:], wg_t[:, j, ko, :],
                            x_sb[:, ko, o:o + sz],
                            start=(ko == 0), stop=(ko == KO - 1),
                        )
                for ko in range(KO):
                    for i, (o, sz) in enumerate(chunks):
                        nc.tensor.matmul(
                            pus[i][:], wu_t[:, j, ko, :],
                            x_sb[:, ko, o:o + sz],
                            start=(ko == 0), stop=(ko == KO - 1),
                        )
                for i, (o, sz) in enumerate(chunks):
                    sg = spool.tile([P, sz], _F32, tag="sg", name="sg")
                    nc.scalar.activation(sg[:], pgs[i][:], _ACTF.Sigmoid)
                    nc.vector.tensor_tensor(sg[:], sg[:], pgs[i][:],
                                            _ALU.mult)
                    nc.vector.tensor_tensor(dst[:, fb, o:o + sz], sg[:],
                                            pus[i][:], _ALU.mult)

    def down_phase(a_sb, d_sb, blocks, out_d, scale, split_out=False):
        """Down-projection, activation-stationary (lhs = a[f, tok-block]),
        streaming the [f, H] weights as the moving operand."""
        for tb, (t0, bs) in enumerate(blocks):
            tbs = slice(t0, t0 + bs)
            pds = [mmp.tile([P, 512], _F32, tag=f"p{hc}", bufs=2,
                            name=f"pd{hc}") for hc in range(HC)]
            for fb in range(FBN):
                for hc in range(HC):
                    nc.tensor.matmul(
                        pds[hc][:bs], a_sb[:, fb, tbs],
                        d_sb[:, fb, hc * 512:(hc + 1) * 512],
                        start=(fb == 0), stop=(fb == FBN - 1),
                    )
            o = opool.tile([P, H], _F32, tag="o", bufs=3, name="o")
            for hc in range(HC):
                hs = slice(hc * 512, (hc + 1) * 512)
                if scale:
                    nc.vector.tensor_scalar_mul(o[:bs, hs], pds[hc][:bs],
                                                wr_sb[:bs, tb:tb + 1])
                else:
                    nc.vector.tensor_copy(o[:bs, hs], pds[hc][:bs])
                if split_out:
                    eng = nc.scalar if hc % 2 == 0 else nc.sync
                    eng.dma_start(out_d[tbs, hs], o[:bs, hs])
            if not split_out:
                eng = nc.scalar if tb % 2 == 0 else nc.sync
                eng.dma_start(out_d[tbs, :], o[:bs, :])

    r_chunks = [(0, 512), (512, C - 512)]
    s_chunks = [(0, 512)]
    r_blocks = [(i * P, P) for i in range(4)] + [(512, C - 512)]
    s_blocks = [(i * P, P) for i in range(SB)]
    gu_phase(wg_d, wu_d, xg_sb, r_chunks, aT, after_group={
        1: lambda: nc.scalar.dma_start(dw_sb[:], dw_d[:]),
        2: lambda: nc.scalar.dma_start(xs_sb[:], xs_d[:]),
    })
    down_phase(aT, dw_sb, r_blocks, outr_d, scale=True)
    # sd reuses dw's SBUF slot: its DMA waits for the routed down phase's
    # last dw read, landing well before the shared down phase needs it
    sd_sb = dpool.tile([P, FBN, H], _BF16, tag="dwn", name="sd_sb")
    nc.scalar.dma_start(sd_sb[:], sd_d[:])
    gu_phase(sg_d, su_d, xs_sb, s_chunks, asT)
    down_phase(asT, sd_sb, s_blocks, outs_d, scale=False, split_out=True)


def build_program():
    nc = bacc.Bacc("TRN2", target_bir_lowering=False, debug=False)
    with tile.TileContext(nc) as tc:
        with ExitStack() as ctx:
            _build_body(ctx, tc)
    nc.compile()
    return nc


def _get_nc():
    global _CACHED_NC
    if _CACHED_NC is None:
        _CACHED_NC = build_program()
    return _CACHED_NC


def _route(h, gate_weight):
    """Mirror of the reference's softmax + group-limited top-2, numpy fp32."""
    logits = (h @ gate_weight.T).astype(F32)
    m = logits.max(-1, keepdims=True)
    e = np.exp(logits - m)
    scores = e / e.sum(-1, keepdims=True)                     # [T, N]
    E = NEXP // N_GROUP
    gs = scores.reshape(T, N_GROUP, E).max(-1)                # [T, G]
    gidx = np.argsort(-gs, axis=1, kind="stable")[:, :TOPK_GROUP]
    gmask = np.zeros((T, N_GROUP), F32)
    np.put_along_axis(gmask, gidx, 1.0, axis=1)
    masked = np.where(np.repeat(gmask, E, axis=1) > 0, scores, 0.0)
    ti = np.argsort(-masked, axis=1, kind="stable")[:, :TOP_K]  # [T, K]
    tw = np.take_along_axis(masked, ti, axis=1)               # [T, K]
    return tw, ti


def _wslab(w, half=None):
    """[F', H] row-major weight -> [P, FBN, KO, P] bf16 lhsT layout."""
    m = w if half is None else w[half * F:(half + 1) * F]
    return np.ascontiguousarray(
        m.reshape(FBN, P, KO, P).transpose(3, 0, 2, 1).astype(BF16))


def _dslab(w, half=None):
    """[H, F'] down weight -> [P, FBN, H] bf16 (f-inner, fb, h') layout."""
    m = w if half is None else w[:, half * F:(half + 1) * F]
    return np.ascontiguousarray(
        m.T.reshape(FBN, P, H).transpose(1, 0, 2).astype(BF16))


def _xslab(hT, pad_to):
    """[H, t] f32 column-slice of tokens -> [P, KO, pad_to] bf16."""
    t = hT.shape[1]
    out = np.zeros((P, KO, pad_to), BF16)
    out[:, :, :t] = hT.reshape(KO, P, t).transpose(1, 0, 2).astype(BF16)
    return out


def prepare(inputs):
    h = np.asarray(inputs["hidden_states"], F32).reshape(T, H)
    hT = np.ascontiguousarray(h.T)                            # [H, T]
    tw, ti = _route(h, np.asarray(inputs["gate_weight"], F32))

    gate_w = np.asarray(inputs["gate_w"], F32)
    up_w = np.asarray(inputs["up_w"], F32)
    down_w = np.asarray(inputs["down_w"], F32)

    # shared-expert shards, built once and referenced by multiple cores
    sgh = [_wslab(np.asarray(inputs["sh_gate_w"], F32), hn) for hn in (0, 1)]
    suh = [_wslab(np.asarray(inputs["sh_up_w"], F32), hn) for hn in (0, 1)]
    sdh = [_dslab(np.asarray(inputs["sh_down_w"], F32), hn) for hn in (0, 1)]
    xsq = [_xslab(hT[:, q * TS:(q + 1) * TS], TS) for q in range(4)]

    in_maps, gathers, overflows = [], [], []
    for n in range(NEXP):
        sel = np.nonzero(ti == n)
        idx = sel[0]                                          # token ids
        w = tw[sel[0], sel[1]]                                # combine weights
        if len(idx) > C:
            overflows.append((n, idx[C:], w[C:]))
            idx, w = idx[:C], w[:C]
        gathers.append((idx, len(idx)))
        wr = np.zeros(CB * P, F32)
        wr[:len(idx)] = w
        in_maps.append({
            "xg": _xslab(hT[:, idx], C),
            "xs": xsq[n // 2],
            "wg": _wslab(gate_w[n]),
            "wu": _wslab(up_w[n]),
            "sg": sgh[n % 2],
            "su": suh[n % 2],
            "dw": _dslab(down_w[n]),
            "sd": sdh[n % 2],
            "wr": np.ascontiguousarray(wr.reshape(CB, P).T),
        })
    return in_maps, gathers, overflows


def _silu(x):
    return x / (1.0 + np.exp(-x))


def run(inputs, trace=False, **kwargs):
    nc = _get_nc()
    in_maps, gathers, overflows = prepare(inputs)
    res = run_bass_kernel_spmd(
        nc, in_maps, core_ids=list(range(NEXP)), trace=trace, **kwargs
    )
    out = np.empty((T, H), F32)
    for q in range(4):
        out[q * TS:(q + 1) * TS] = (res.results[2 * q]["outs"]
                                    + res.results[2 * q + 1]["outs"])
    for n in range(NEXP):
        idx, cnt = gathers[n]
        out[idx] += res.results[n]["outr"][:cnt]
    for n, idx, w in overflows:   # exact host fallback, normally empty
        x = np.asarray(inputs["hidden_states"], F32).reshape(T, H)[idx]
        g = x @ np.asarray(inputs["gate_w"][n], F32).T
        u = x @ np.asarray(inputs["up_w"][n], F32).T
        out[idx] += ((_silu(g) * u) @ np.asarray(inputs["down_w"][n], F32).T
                     * w[:, None])
    return out.reshape(B, S, H), res


def kernel(**inputs):
    out, _ = run(inputs)
    return out
